# revision 4
# baseline (speedup 1.0000x reference)
"""MoE-routed DeepQNetwork kernel for 8x Trainium2 NeuronCores.

Problem: B=65536 rows, each routed to one of E=8 expert MLPs
(256 -> 64 -> 64 -> 64 -> 64 -> 64 -> 18, ReLU between layers).

Strategy v7 (expert-per-core, greedy lag-1 pipeline, early HAM boost):
  E == NCORES and the routing is near-uniform (~8192 rows/expert), so core k
  owns ALL rows of expert k, padded to a uniform C = nb*512 columns. Every
  core runs the same static program with a SINGLE expert's weights (~180 KB).

  Trace-driven changes vs v6 (baseline 46.8us):
  - Output stores moved from gpsimd SWDGE to the scalar HWDGE ring: the
    program epilogue waited on DMASW* semaphores plus a ~1.9us Pool drain,
    stretching the post-work teardown to ~9us.
  - A gap-free ~3.8us dummy-matmul burst right after the preamble keeps the
    PE busy through one full HAM activity window, so the clock boost
    (1.2 -> 2.4 GHz) fires at ~10.5us instead of ~24us; every real matmul
    runs warm.
  - Lag-1 greedy schedule: wave w emits L2..L6 (+store) of unit w-1 BEFORE
    L1 of unit w, so the PE fills x-DMA sem-lag windows with deep-layer
    work and the post-last-chunk drain tail is a single unit (~2us), not a
    6-deep wavefront.
  - Activations (PSUM fp32 -> SBUF fp16, 1 elem/cycle/partition on TRN2)
    are nearly as expensive as the matmuls; they are assigned to DVE vs ACT
    per-instruction by a static cost model that keeps both engines level.
  - L1 issue order per pair is (chunk, block) interleaved so the two
    64-wide column-group matmuls of a pair stream concurrently.

  Host: unsort the fp16 outputs back to row order, cast to fp32.
"""

import math
import os

import numpy as np

E = 8
D = 256
H = 64
A = 18
NCORES = 8
BLK = 512  # rows per block (matmul moving-operand free dim / PSUM bank cols)
NWARM = 9  # gap-free PE warm-up matmuls (~3.8us at 1.2 GHz) for the HAM boost

# per-core weight tile [128, WCOLS] fp16 column layout:
#   [0:64)    W1 chunk0 (input dims 0:128)
#   [64:128)  W1 chunk1 (input dims 128:256)
#   [128+128*li : 256+128*li) for li in 0..3: layer 2+li block-diag [128,128]
#             ([0:64,0:64] = W, [64:128,64:128] = W)
#   [640:704) W6 block-diag: [0:64, 0:18] = W6, [64:128, 32:50] = W6
WCOLS = 704

_PROGRAM_CACHE: dict = {}
LAST_RESULTS = None  # test harness can read timing/profile info from here


def _build_program(nb: int):
    """Build the SPMD bass program for nb 512-row blocks per core."""
    import concourse.mybir as mybir
    import concourse.tile as tile
    from concourse import bacc

    f32 = mybir.dt.float32
    f16 = mybir.dt.float16
    Relu = mybir.ActivationFunctionType.Relu
    add = mybir.AluOpType.add
    amax = mybir.AluOpType.max

    npair = nb // 2
    lone = nb % 2  # trailing unpaired block
    ndbl = (npair + 1) // 2  # two-pair groups (last may hold one pair)
    ngrp = ndbl + lone  # output column groups in yt

    nc = bacc.Bacc("TRN2")
    xall = nc.declare_dram_parameter(
        "xall", [128, npair * 2048 + lone * 1024], f16, isOutput=False
    )
    wt = nc.declare_dram_parameter("wt", [128, WCOLS], f16, isOutput=False)
    # bias cols 0:5 = b1..b5 (rows 0:64 == rows 64:128); col 5 = b6 at rows
    # 0:18 / 32:50 / 64:82 / 96:114
    bias = nc.declare_dram_parameter("bias", [128, 6], f32, isOutput=False)
    yt = nc.declare_dram_parameter("yt", [128, ngrp * BLK], f16, isOutput=True)

    # static DVE/ACT load balancer for the PSUM-drain activations
    eng_debt = [0.0, 0.0]  # [DVE, ACT]

    with tile.TileContext(nc) as tc:
        with (
            tc.tile_pool(name="wpool", bufs=1) as wpool,
            tc.tile_pool(name="xpool", bufs=2) as xpool,
            tc.tile_pool(name="hpool", bufs=1) as hpool,
            tc.tile_pool(name="opool", bufs=2) as opool,
            # PSUM budget (8 banks): p1pool 2 x [128,512] = 2 banks
            # (warm-ups, L1 pairs, L6 po), pmpool 3 x [128,1024] = 6 banks
            tc.tile_pool(name="p1pool", bufs=2, space="PSUM") as p1pool,
            tc.tile_pool(name="pmpool", bufs=3, space="PSUM") as pmpool,
        ):
            def dbl_pairs(d):
                return [q for q in (2 * d, 2 * d + 1) if q < npair]

            # ---- PE warm-up source (memset, no DMA dependence)
            warm_src = wpool.tile([1, BLK], f16, name="warm_src", tag="ws", bufs=1)
            nc.vector.memset(warm_src[:, :], 0.0)

            # ---- DMA issue. Weights+bias on the scalar HW-DGE ring first
            # (receipt is fast before the x flood saturates HBM); all x on
            # the sync HW-DGE ring in consumption order; outputs ride the
            # scalar ring (idle after the weights) -- NOT gpsimd SWDGE,
            # whose drain+semaphores dominated the epilogue.
            w_sb = wpool.tile([128, WCOLS], f16, name="w_sb", tag="w", bufs=1)
            nc.scalar.dma_start(out=w_sb[:, :], in_=wt[:, :])
            bias_sb = wpool.tile([128, 6], f32, name="bias_sb", tag="bias", bufs=1)
            nc.scalar.dma_start(out=bias_sb[:, :], in_=bias[:, :])

            # pair0 as two [128,1024] half-chunks (first-needed first), then
            # pair1, then 1MB two-pair chunks; the lone block arrives last
            # (it is also processed last, so the pipeline drain is short).
            p0 = []
            for i in (0, 1):
                t = xpool.tile([128, 1024], f16, tag=f"x0h{i}", name=f"x0h{i}", bufs=1)
                nc.sync.dma_start(out=t[:, :], in_=xall[:, i * 1024 : (i + 1) * 1024])
                p0.append(t)
            xc1 = None
            if npair > 1:
                xc1 = xpool.tile([128, 2048], f16, tag="xc1", name="xc_1", bufs=1)
                nc.sync.dma_start(out=xc1[:, :], in_=xall[:, 2048:4096])
            xds: list = [None] * ndbl
            for dd in range(1, ndbl):
                w = len(dbl_pairs(dd)) * 2048
                xd = xpool.tile([128, w], f16, tag=f"xd{dd}", name=f"xd_{dd}", bufs=1)
                nc.sync.dma_start(
                    out=xd[:, :], in_=xall[:, 2 * dd * 2048 : 2 * dd * 2048 + w]
                )
                xds[dd] = xd
            xl = None
            if lone:
                xl = xpool.tile([128, 1024], f16, tag="xl", name="xlone", bufs=1)
                nc.sync.dma_start(
                    out=xl[:, :], in_=xall[:, npair * 2048 : npair * 2048 + 1024]
                )

            def x_rhs(p, blk, c):
                if p == 0:
                    return p0[c][:, blk * BLK : (blk + 1) * BLK]
                if p == 1:
                    return xc1[:, c * 1024 + blk * BLK : c * 1024 + (blk + 1) * BLK]
                xd = xds[p // 2]
                off = (p % 2) * 2048 + c * 1024 + blk * BLK
                return xd[:, off : off + BLK]

            # ---- PE warm-up burst: gap-free writes cycling the p1 PSUM
            # ring, never read.  ~3.8us of sustained PE activity releases
            # the HAM clock gate right as the first x chunk lands.
            for i in range(NWARM):
                warm_ps = p1pool.tile([128, BLK], f32, tag="p1", name=f"warm_{i}")
                nc.tensor.matmul(
                    out=warm_ps[:, :],
                    lhsT=warm_src[0:1, 0:128],
                    rhs=warm_src[0:1, :],
                    start=True,
                    stop=True,
                )

            def act(out_ap, in_ap, bias_ap, relu, fd):
                # assign to DVE or ACT, whichever finishes first under a
                # static cost model ((fixed + FD) / clock, ns)
                cost_v = (120.0 + fd) / 0.96 + 90.0
                cost_s = (172.0 + fd) / 1.2 + 120.0
                if eng_debt[0] + cost_v <= eng_debt[1] + cost_s:
                    eng_debt[0] += cost_v
                    if relu:
                        nc.vector.tensor_scalar(
                            out_ap, in_ap, bias_ap, 0.0, op0=add, op1=amax
                        )
                    else:
                        nc.vector.tensor_scalar(out_ap, in_ap, bias_ap, None, op0=add)
                else:
                    eng_debt[1] += cost_s
                    if relu:
                        nc.scalar.activation(out_ap, in_ap, Relu, bias=bias_ap)
                    else:
                        nc.scalar.add(out_ap, in_ap, bias_ap)

            # h storage: layer 1 per pair (+ lone), layers 2-5 per dbl (+ lone)
            h1s = [None] * npair
            hdbl = {li: [None] * ndbl for li in (2, 3, 4, 5)}
            hlon = {}

            def emit_s1(u):
                """Layer 1 for unit u: per pair, (chunk, block)-interleaved
                matmuls so the two 64-wide column groups stream concurrently."""
                if u == -1:
                    phl = p1pool.tile([128, BLK], f32, tag="p1", name="ph1_l")
                    for c in (0, 1):
                        nc.tensor.matmul(
                            out=phl[0:64, :],
                            lhsT=w_sb[:, c * H : (c + 1) * H],
                            rhs=xl[:, c * BLK : (c + 1) * BLK],
                            start=(c == 0),
                            stop=(c == 1),
                        )
                    hl = hpool.tile([64, BLK], f16, tag="hl1", name="h1_l", bufs=1)
                    act(hl[:, :], phl[0:64, :], bias_sb[0:64, 0:1], True, BLK)
                    hlon[1] = hl
                    return
                for p in dbl_pairs(u):
                    ph = p1pool.tile([128, BLK], f32, tag="p1", name=f"ph1_{p}")
                    for c in (0, 1):
                        for blk, colr in ((0, slice(0, 64)), (1, slice(64, 128))):
                            nc.tensor.matmul(
                                out=ph[colr, :],
                                lhsT=w_sb[:, c * H : (c + 1) * H],
                                rhs=x_rhs(p, blk, c),
                                start=(c == 0),
                                stop=(c == 1),
                            )
                    h1 = hpool.tile(
                        [128, BLK], f16, tag=f"h1_{p}", name=f"h1_{p}", bufs=1
                    )
                    act(h1[:, :], ph[:, :], bias_sb[:, 0:1], True, BLK)
                    h1s[p] = h1

            def emit_mid(li, u):
                # layer li in 2..5: [64 -> 64] block-diag
                wc = 128 + (li - 2) * 128
                if u == -1:
                    prev = hlon[li - 1]
                    ph = pmpool.tile([128, 1024], f32, tag="pm", name=f"ph{li}_l")
                    nc.tensor.matmul(
                        out=ph[0:64, 0:BLK],
                        lhsT=w_sb[0:64, wc : wc + 64],
                        rhs=prev[:, :],
                        start=True,
                        stop=True,
                    )
                    hl = hpool.tile(
                        [64, BLK], f16, tag=f"hl{li}", name=f"h{li}_l", bufs=1
                    )
                    act(hl[:, :], ph[0:64, 0:BLK], bias_sb[0:64, li - 1 : li], True, BLK)
                    hlon[li] = hl
                    return
                ph = pmpool.tile([128, 1024], f32, tag="pm", name=f"ph{li}_{u}")
                w = len(dbl_pairs(u)) * BLK
                for k, p in enumerate(dbl_pairs(u)):
                    co = k * BLK
                    rhs = h1s[p][:, :] if li == 2 else hdbl[li - 1][u][:, co : co + BLK]
                    nc.tensor.matmul(
                        out=ph[:, co : co + BLK],
                        lhsT=w_sb[:, wc : wc + 128],
                        rhs=rhs,
                        start=True,
                        stop=True,
                    )
                h = hpool.tile([128, w], f16, tag=f"h{li}_{u}", name=f"h{li}_{u}", bufs=1)
                act(h[:, :], ph[:, 0:w], bias_sb[:, li - 1 : li], True, w)
                hdbl[li][u] = h

            def emit_s6(u):
                # L6 [64 -> 18]: group u = pairs 2u (PSUM rows 0:64) and
                # 2u+1 (rows 64:128) -- two column groups, concurrent;
                # u == -1 = lone block (rows 0:32, output column group ndbl)
                if u == -1:
                    po = p1pool.tile([128, BLK], f32, tag="p1", name="po_l")
                    nc.tensor.matmul(
                        out=po[0:32, :],
                        lhsT=w_sb[0:64, 640:672],
                        rhs=hlon[5][:, :],
                        start=True,
                        stop=True,
                    )
                    o = opool.tile([32, BLK], f16, tag="og", name="o_l", bufs=2)
                    act(o[:, :], po[0:32, :], bias_sb[0:32, 5:6], False, BLK)
                    nc.scalar.dma_start(
                        out=yt[0:32, ndbl * BLK : (ndbl + 1) * BLK], in_=o[:, :]
                    )
                    return
                pairs = dbl_pairs(u)
                rows = 64 * len(pairs)
                po = p1pool.tile([128, BLK], f32, tag="p1", name=f"po_{u}")
                for k, q in enumerate(pairs):
                    nc.tensor.matmul(
                        out=po[64 * k : 64 * (k + 1), :],
                        lhsT=w_sb[:, 640:704],
                        rhs=hdbl[5][u][:, k * BLK : (k + 1) * BLK],
                        start=True,
                        stop=True,
                    )
                o = opool.tile([rows, BLK], f16, tag="og", name=f"o_{u}", bufs=2)
                act(o[:, :], po[0:rows, :], bias_sb[0:rows, 5:6], False, BLK)
                nc.scalar.dma_start(
                    out=yt[0:rows, u * BLK : (u + 1) * BLK], in_=o[:, :]
                )

            def emit_deep(u):
                for li in (2, 3, 4, 5):
                    emit_mid(li, u)
                emit_s6(u)

            # ---- lag-1 greedy pipeline: units in x-arrival order, lone
            # last.  Wave w runs the deep layers (L2..L6 + store) of unit
            # w-1 FIRST -- work whose inputs are already on-chip -- then L1
            # of unit w, which may wait on the x-chunk semaphore.  The x
            # sem lag is absorbed by the deep work; the drain tail after
            # the last chunk is a single unit's deep layers.
            units = list(range(ndbl)) + ([-1] if lone else [])
            nunits = len(units)
            for wave in range(nunits + 1):
                if wave >= 1:
                    emit_deep(units[wave - 1])
                if wave < nunits:
                    emit_s1(units[wave])

    nc.compile()
    return nc


def _get_program(nb: int):
    if nb not in _PROGRAM_CACHE:
        _PROGRAM_CACHE[nb] = _build_program(nb)
    return _PROGRAM_CACHE[nb]


def _prepare(state, rm_state, W1, b1, W2, b2, W3, b3, W4, b4, W5, b5, W6, b6):
    state = np.ascontiguousarray(np.asarray(state, dtype=np.float32))
    rm = np.asarray(rm_state).reshape(-1).astype(np.int64)
    Ws = [np.asarray(w, dtype=np.float32) for w in (W1, W2, W3, W4, W5, W6)]
    bs = [np.asarray(b, dtype=np.float32) for b in (b1, b2, b3, b4, b5, b6)]
    B = state.shape[0]
    X = state.reshape(B, D)

    # ---- host-side routing: all rows of expert k go to core k
    order = np.argsort(rm, kind="stable")
    counts = np.bincount(rm, minlength=E)
    nb = max(2, math.ceil(counts.max() / BLK))
    C = nb * BLK
    npair = nb // 2
    lone = nb % 2
    ndbl = (npair + 1) // 2
    ngrp = ndbl + lone
    csum = np.zeros(E, dtype=np.int64)
    csum[1:] = np.cumsum(counts)[:-1]
    sorted_expert = rm[order]
    pos_sorted = sorted_expert * C + (np.arange(B) - csum[sorted_expert])

    Xp = np.zeros((E * C, D), np.float16)
    Xp[pos_sorted] = X[order].astype(np.float16)

    W16 = [w.astype(np.float16) for w in Ws]

    in_maps = []
    for core in range(E):
        xt = Xp[core * C : (core + 1) * C].T  # [D, C] fp16 view
        # pairs: interleave the two 128-row halves per pair -> [128, 2048]
        parts = [
            xt[:, : npair * 1024]
            .reshape(2, 128, npair, 2 * BLK)
            .transpose(1, 2, 0, 3)
            .reshape(128, npair * 4 * BLK)
        ]
        if lone:
            xlh = xt[:, npair * 1024 :].reshape(2, 128, BLK)
            parts.append(xlh[0])
            parts.append(xlh[1])
        xint = np.ascontiguousarray(np.concatenate(parts, axis=1))

        wh = np.zeros((128, WCOLS), np.float16)
        wh[:, 0:H] = W16[0][core, 0:128, :]
        wh[:, H : 2 * H] = W16[0][core, 128:256, :]
        for li in range(4):
            wc = 128 + li * 128
            wh[0:64, wc : wc + H] = W16[li + 1][core]
            wh[64:128, wc + H : wc + 128] = W16[li + 1][core]
        wh[0:64, 640 : 640 + A] = W16[5][core]
        wh[64:128, 672 : 672 + A] = W16[5][core]

        bh = np.zeros((128, 6), np.float32)
        for li in range(5):
            bh[0:64, li] = bs[li][core]
            bh[64:128, li] = bs[li][core]
        for r0 in (0, 32, 64, 96):
            bh[r0 : r0 + A, 5] = bs[5][core]

        in_maps.append({"xall": xint, "wt": wh, "bias": bh})

    meta = dict(
        B=B,
        C=C,
        nb=nb,
        npair=npair,
        lone=lone,
        ndbl=ndbl,
        ngrp=ngrp,
        order=order,
        pos_sorted=pos_sorted,
    )
    return in_maps, meta


def _finalize(results, meta):
    """results: list (per core) of dicts with 'yt' [128, ngrp*BLK] fp16."""
    B, C, nb, npair, lone, ndbl = (
        meta[k] for k in ("B", "C", "nb", "npair", "lone", "ndbl")
    )
    Yp = np.zeros((E * C, A), np.float32)
    for core in range(E):
        ytc = results[core]["yt"].astype(np.float32)
        for g in range(ndbl):
            cols = slice(g * BLK, (g + 1) * BLK)
            for k, q in enumerate((2 * g, 2 * g + 1)):
                if q >= npair:
                    continue
                dst = core * C + 2 * q * BLK
                r0 = 64 * k
                Yp[dst : dst + BLK] = ytc[r0 : r0 + A, cols].T
                Yp[dst + BLK : dst + 2 * BLK] = ytc[r0 + 32 : r0 + 32 + A, cols].T
        if lone:
            cols = slice(ndbl * BLK, (ndbl + 1) * BLK)
            dst = core * C + (nb - 1) * BLK
            Yp[dst : dst + BLK] = ytc[0:A, cols].T

    y = np.zeros((B, A), np.float32)
    y[meta["order"]] = Yp[meta["pos_sorted"]]
    return y


def kernel(state, rm_state, W1, b1, W2, b2, W3, b3, W4, b4, W5, b5, W6, b6):
    global LAST_RESULTS
    from concourse.bass_utils import run_bass_kernel_spmd

    in_maps, meta = _prepare(
        state, rm_state, W1, b1, W2, b2, W3, b3, W4, b4, W5, b5, W6, b6
    )
    nc = _get_program(meta["nb"])
    trace = bool(os.environ.get("KERNEL_TRACE"))
    res = run_bass_kernel_spmd(nc, in_maps, core_ids=list(range(NCORES)), trace=trace)
    LAST_RESULTS = res
    return _finalize(res.results, meta)


# revision 6
# speedup vs baseline: 1.5208x; 1.5208x over previous
"""MoE-routed DeepQNetwork kernel for 8x Trainium2 NeuronCores.

Problem: B=65536 rows, each routed to one of E=8 expert MLPs
(256 -> 64 -> 64 -> 64 -> 64 -> 64 -> 18, ReLU between layers).

Strategy v8 (expert-per-core, staggered wavefront, short tail):
  E == NCORES and the routing is near-uniform (~8192 rows/expert), so core k
  owns ALL rows of expert k, padded to npair 1024-row pairs plus an optional
  short remainder block of r <= 512 rows.  Every core runs the same static
  program with a SINGLE expert's weights (~180 KB).

  Measured constraints driving the design (from perfetto traces):
  - exec_time ~= (last output-store data lands) + ~2.8us: the program
    epilogue (DMA-sem waits, engine barriers, notification) is a fixed
    ~8.7us after the last store, and the first ~5.9us of preamble is
    excluded from the measured window.  So the objective is purely to
    finish the last store as early as possible.
  - The PE is in-order: consecutive dependent stages of one unit serialize
    mm -> act -> mm (act = PSUM fp32 drain at 1 elem/cycle/partition on
    DVE/ACT; TRN2 has no 16-bit PSUM).  A staggered wavefront (stage s of
    unit u in wave u+s-1, deepest stage first within a wave) keeps 5-6
    independent units in flight so acts overlap other units' matmuls.
  - x streams at ~400 GB/s on the sync HWDGE ring but each chunk's
    completion semaphore lags its data by 1.5-3us (HBM receipt under
    load); the last x sem fires ~21us in.  After that only the last
    units' stage chains remain; their latency is act-bound, so the last
    pair-group and the remainder block run their stages in 256-column
    sub-segments to halve the act->mm ping-pong latency.
  - The PE idles at 1.2 GHz until ~3.4us of *sustained* activity (HAM
    clock gate).  A gap-free warm-up burst of dummy matmuls bridges
    program start to the first x-chunk semaphore, so real matmuls run at
    2.4 GHz from the start.
  - Output stores ride the scalar HWDGE ring (idle after weights); gpsimd
    SWDGE stores added Pool-drain + DMASW waits to the epilogue.
  - Activations are assigned DVE vs ACT per-instruction by a static cost
    model ((fixed + FD)/clock) keeping both engines level.

  Host: unsort the fp16 outputs back to row order, cast to fp32.
"""

import math
import os

import numpy as np

E = 8
D = 256
H = 64
A = 18
NCORES = 8
BLK = 512  # rows per full block (matmul free dim / PSUM bank cols)
NWARM = 7  # gap-free PE warm-up matmuls (~3us at 1.2 GHz) for the HAM boost

# per-core weight tile [128, WCOLS] fp16 column layout:
#   [0:64)    W1 chunk0 (input dims 0:128)
#   [64:128)  W1 chunk1 (input dims 128:256)
#   [128+128*li : 256+128*li) for li in 0..3: layer 2+li block-diag [128,128]
#             ([0:64,0:64] = W, [64:128,64:128] = W)
#   [640:704) W6 block-diag: [0:64, 0:18] = W6, [64:128, 32:50] = W6
WCOLS = 704

_PROGRAM_CACHE: dict = {}
LAST_RESULTS = None  # test harness can read timing/profile info from here


def _build_program(npair: int, rcols: int):
    """SPMD bass program: npair 1024-row pairs + optional rcols remainder."""
    import concourse.mybir as mybir
    import concourse.tile as tile
    from concourse import bacc

    f32 = mybir.dt.float32
    f16 = mybir.dt.float16
    Relu = mybir.ActivationFunctionType.Relu
    add = mybir.AluOpType.add
    amax = mybir.AluOpType.max

    lone = 1 if rcols else 0
    ndbl = (npair + 1) // 2  # two-pair groups (last may hold one pair)
    ngrp = ndbl + lone  # output column groups in yt
    ycols = ndbl * BLK + lone * rcols

    nc = bacc.Bacc("TRN2")
    xall = nc.declare_dram_parameter(
        "xall", [128, npair * 2048 + lone * 2 * rcols], f16, isOutput=False
    )
    wt = nc.declare_dram_parameter("wt", [128, WCOLS], f16, isOutput=False)
    # bias cols 0:5 = b1..b5 (rows 0:64 == rows 64:128); col 5 = b6 at rows
    # 0:18 / 32:50 / 64:82 / 96:114
    bias = nc.declare_dram_parameter("bias", [128, 6], f32, isOutput=False)
    yt = nc.declare_dram_parameter("yt", [128, ycols], f16, isOutput=True)

    eng_debt = [0.0, 0.0]  # [DVE, ACT] static act load balancer

    def dbl_pairs(d):
        return [q for q in (2 * d, 2 * d + 1) if q < npair]

    tail_dbl = ndbl - 1  # last two-pair unit: 256-col stage sub-segments

    def segs(p):
        # column sub-segments for pair p's stages (tail pairs split in two)
        if p in dbl_pairs(tail_dbl):
            return ((0, 256), (256, 512))
        return ((0, BLK),)

    with tile.TileContext(nc) as tc:
        with (
            tc.tile_pool(name="wpool", bufs=1) as wpool,
            tc.tile_pool(name="xpool", bufs=2) as xpool,
            tc.tile_pool(name="hpool", bufs=1) as hpool,
            tc.tile_pool(name="opool", bufs=3) as opool,
            # PSUM budget (8 banks): p1pool 3 x [128,512] (warm-ups, L1,
            # L6) + pmpool 5 x [128,512] (per-pair mid stages)
            tc.tile_pool(name="p1pool", bufs=3, space="PSUM") as p1pool,
            tc.tile_pool(name="pmpool", bufs=5, space="PSUM") as pmpool,
        ):
            # ---- PE warm-up source (memset, no DMA dependence)
            warm_src = wpool.tile([1, BLK], f16, name="warm_src", tag="ws", bufs=1)
            nc.vector.memset(warm_src[:, :], 0.0)

            # ---- DMA issue.  Weights+bias on the scalar HW-DGE ring
            # first; all x on the sync ring in consumption order; output
            # stores ride the scalar ring later.
            w_sb = wpool.tile([128, WCOLS], f16, name="w_sb", tag="w", bufs=1)
            nc.scalar.dma_start(out=w_sb[:, :], in_=wt[:, :])
            bias_sb = wpool.tile([128, 6], f32, name="bias_sb", tag="bias", bufs=1)
            nc.scalar.dma_start(out=bias_sb[:, :], in_=bias[:, :])

            # x chunks: pair0 as two 1024-col halves, pair1, then 1MB
            # two-pair chunks; the tail dbl's pairs get their own chunks
            # (finer completion sems for the drain); remainder last.
            p0 = []
            for i in (0, 1):
                t = xpool.tile([128, 1024], f16, tag=f"x0h{i}", name=f"x0h{i}", bufs=1)
                nc.sync.dma_start(out=t[:, :], in_=xall[:, i * 1024 : (i + 1) * 1024])
                p0.append(t)
            xc1 = None
            if npair > 1:
                xc1 = xpool.tile([128, 2048], f16, tag="xc1", name="xc_1", bufs=1)
                nc.sync.dma_start(out=xc1[:, :], in_=xall[:, 2048:4096])
            xds: dict = {}
            for dd in range(1, ndbl):
                prs = dbl_pairs(dd)
                if dd == tail_dbl:
                    for q in prs:
                        t = xpool.tile(
                            [128, 2048], f16, tag=f"xp{q}", name=f"xp_{q}", bufs=1
                        )
                        nc.sync.dma_start(
                            out=t[:, :], in_=xall[:, q * 2048 : (q + 1) * 2048]
                        )
                        xds[q] = t
                else:
                    w = len(prs) * 2048
                    t = xpool.tile([128, w], f16, tag=f"xd{dd}", name=f"xd_{dd}", bufs=1)
                    nc.sync.dma_start(
                        out=t[:, :], in_=xall[:, 2 * dd * 2048 : 2 * dd * 2048 + w]
                    )
                    for q in prs:
                        xds[q] = (t, (q % 2) * 2048)
            xl = None
            if lone:
                xl = xpool.tile([128, 2 * rcols], f16, tag="xl", name="xlone", bufs=1)
                nc.sync.dma_start(
                    out=xl[:, :],
                    in_=xall[:, npair * 2048 : npair * 2048 + 2 * rcols],
                )

            def x_rhs(p, blk, c, c0, c1):
                # columns [c0:c1) of contraction chunk c of block blk of pair p
                lo = c * 1024 + blk * BLK + c0
                if p == 0:
                    return p0[c][:, blk * BLK + c0 : blk * BLK + c1]
                if p == 1:
                    return xc1[:, lo : lo + (c1 - c0)]
                ent = xds[p]
                if isinstance(ent, tuple):
                    t, off = ent
                    return t[:, off + lo : off + lo + (c1 - c0)]
                return ent[:, lo : lo + (c1 - c0)]

            # ---- PE warm-up burst (gap-free, cycles the p1 ring, never read)
            for i in range(NWARM):
                wps = p1pool.tile([128, BLK], f32, tag="p1", name=f"warm_{i}")
                nc.tensor.matmul(
                    out=wps[:, :],
                    lhsT=warm_src[0:1, 0:128],
                    rhs=warm_src[0:1, :],
                    start=True,
                    stop=True,
                )

            def act(out_ap, in_ap, bias_ap, relu, fd):
                cost_v = (120.0 + fd) / 0.96 + 90.0
                cost_s = (172.0 + fd) / 1.2 + 120.0
                if eng_debt[0] + cost_v <= eng_debt[1] + cost_s:
                    eng_debt[0] += cost_v
                    if relu:
                        nc.vector.tensor_scalar(
                            out_ap, in_ap, bias_ap, 0.0, op0=add, op1=amax
                        )
                    else:
                        nc.vector.tensor_scalar(out_ap, in_ap, bias_ap, None, op0=add)
                else:
                    eng_debt[1] += cost_s
                    if relu:
                        nc.scalar.activation(out_ap, in_ap, Relu, bias=bias_ap)
                    else:
                        nc.scalar.add(out_ap, in_ap, bias_ap)

            # h tiles: h1[(p, s0)], hmid[(li, p, s0)]; lone keyed p='L'
            h1s: dict = {}
            hmid: dict = {}

            def emit_s1(u):
                if u == -1:
                    ph = p1pool.tile([128, BLK], f32, tag="p1", name="ph1_L")
                    for c in (0, 1):
                        nc.tensor.matmul(
                            out=ph[0:64, 0:rcols],
                            lhsT=w_sb[:, c * H : (c + 1) * H],
                            rhs=xl[:, c * rcols : (c + 1) * rcols],
                            start=(c == 0),
                            stop=(c == 1),
                        )
                    hl = hpool.tile([64, rcols], f16, tag="h1L", name="h1_L", bufs=1)
                    act(hl[:, :], ph[0:64, 0:rcols], bias_sb[0:64, 0:1], True, rcols)
                    h1s[("L", 0)] = hl
                    return
                for p in dbl_pairs(u):
                    # one PSUM tile (= one bank) per column segment: an
                    # act draining one segment must never share a bank
                    # with in-flight matmuls or the other segment's act
                    phs = {
                        s0: p1pool.tile([128, BLK], f32, tag="p1", name=f"ph1_{p}_{s0}")
                        for (s0, s1) in segs(p)
                    }
                    # (chunk, segment, block): adjacent block matmuls hit
                    # different PE column groups and stream concurrently
                    for c in (0, 1):
                        for (s0, s1) in segs(p):
                            for blk, colr in ((0, slice(0, 64)), (1, slice(64, 128))):
                                nc.tensor.matmul(
                                    out=phs[s0][colr, 0 : s1 - s0],
                                    lhsT=w_sb[:, c * H : (c + 1) * H],
                                    rhs=x_rhs(p, blk, c, s0, s1),
                                    start=(c == 0),
                                    stop=(c == 1),
                                )
                    for (s0, s1) in segs(p):
                        h1 = hpool.tile(
                            [128, s1 - s0],
                            f16,
                            tag=f"h1_{p}_{s0}",
                            name=f"h1_{p}_{s0}",
                            bufs=1,
                        )
                        act(
                            h1[:, :],
                            phs[s0][:, 0 : s1 - s0],
                            bias_sb[:, 0:1],
                            True,
                            s1 - s0,
                        )
                        h1s[(p, s0)] = h1

            def emit_mid(li, u):
                wc = 128 + (li - 2) * 128
                if u == -1:
                    prev = h1s[("L", 0)] if li == 2 else hmid[(li - 1, "L", 0)]
                    ph = pmpool.tile([128, BLK], f32, tag="pm", name=f"ph{li}_L")
                    nc.tensor.matmul(
                        out=ph[0:64, 0:rcols],
                        lhsT=w_sb[0:64, wc : wc + 64],
                        rhs=prev[:, :],
                        start=True,
                        stop=True,
                    )
                    hl = hpool.tile(
                        [64, rcols], f16, tag=f"h{li}L", name=f"h{li}_L", bufs=1
                    )
                    act(
                        hl[:, :],
                        ph[0:64, 0:rcols],
                        bias_sb[0:64, li - 1 : li],
                        True,
                        rcols,
                    )
                    hmid[(li, "L", 0)] = hl
                    return
                for p in dbl_pairs(u):
                    for (s0, s1) in segs(p):
                        w = s1 - s0
                        prev = (
                            h1s[(p, s0)] if li == 2 else hmid[(li - 1, p, s0)]
                        )
                        ph = pmpool.tile(
                            [128, BLK], f32, tag="pm", name=f"ph{li}_{p}_{s0}"
                        )
                        nc.tensor.matmul(
                            out=ph[:, 0:w],
                            lhsT=w_sb[:, wc : wc + 128],
                            rhs=prev[:, :],
                            start=True,
                            stop=True,
                        )
                        h = hpool.tile(
                            [128, w],
                            f16,
                            tag=f"h{li}_{p}_{s0}",
                            name=f"h{li}_{p}_{s0}",
                            bufs=1,
                        )
                        act(h[:, :], ph[:, 0:w], bias_sb[:, li - 1 : li], True, w)
                        hmid[(li, p, s0)] = h

            def emit_s6(u):
                # L6 [64 -> 18]: pairs 2u / 2u+1 -> PSUM rows 0:64 / 64:128
                # (concurrent column groups); store per segment on scalar ring
                if u == -1:
                    po = p1pool.tile([128, BLK], f32, tag="p1", name="po_L")
                    nc.tensor.matmul(
                        out=po[0:32, 0:rcols],
                        lhsT=w_sb[0:64, 640:672],
                        rhs=hmid[(5, "L", 0)][:, :],
                        start=True,
                        stop=True,
                    )
                    o = opool.tile([32, rcols], f16, tag="og", name="o_L")
                    act(o[:, :], po[0:32, 0:rcols], bias_sb[0:32, 5:6], False, rcols)
                    nc.scalar.dma_start(
                        out=yt[0:32, ndbl * BLK : ndbl * BLK + rcols], in_=o[:, :]
                    )
                    return
                pairs = dbl_pairs(u)
                rows = 64 * len(pairs)
                for (s0, s1) in segs(pairs[0]):
                    w = s1 - s0
                    po = p1pool.tile([128, BLK], f32, tag="p1", name=f"po_{u}_{s0}")
                    for k, q in enumerate(pairs):
                        nc.tensor.matmul(
                            out=po[64 * k : 64 * (k + 1), 0:w],
                            lhsT=w_sb[:, 640:704],
                            rhs=hmid[(5, q, s0)][:, :],
                            start=True,
                            stop=True,
                        )
                    o = opool.tile([rows, w], f16, tag="og", name=f"o_{u}_{s0}")
                    act(o[:, :], po[0:rows, 0:w], bias_sb[0:rows, 5:6], False, w)
                    nc.scalar.dma_start(
                        out=yt[0:rows, u * BLK + s0 : u * BLK + s1], in_=o[:, :]
                    )

            def emit_stage(s, u):
                if s == 1:
                    emit_s1(u)
                elif s == 6:
                    emit_s6(u)
                else:
                    emit_mid(s, u)

            # ---- staggered wavefront: stage s of unit i in wave i+s-1,
            # deepest stage first within each wave (oldest dependencies),
            # so an x-sem wait at the wave's trailing L1 never starves the
            # PE of ready deep-layer work.
            units = list(range(ndbl)) + ([-1] if lone else [])
            nunits = len(units)
            lag = {1: 0, 2: 1, 3: 2, 4: 3, 5: 4, 6: 5}
            for wave in range(nunits + lag[6]):
                for s in (6, 5, 4, 3, 2, 1):
                    i = wave - lag[s]
                    if 0 <= i < nunits:
                        emit_stage(s, units[i])

    nc.compile()
    return nc


def _get_program(npair: int, rcols: int):
    key = (npair, rcols)
    if key not in _PROGRAM_CACHE:
        _PROGRAM_CACHE[key] = _build_program(npair, rcols)
    return _PROGRAM_CACHE[key]


def _prepare(state, rm_state, W1, b1, W2, b2, W3, b3, W4, b4, W5, b5, W6, b6):
    state = np.ascontiguousarray(np.asarray(state, dtype=np.float32))
    rm = np.asarray(rm_state).reshape(-1).astype(np.int64)
    Ws = [np.asarray(w, dtype=np.float32) for w in (W1, W2, W3, W4, W5, W6)]
    bs = [np.asarray(b, dtype=np.float32) for b in (b1, b2, b3, b4, b5, b6)]
    B = state.shape[0]
    X = state.reshape(B, D)

    # ---- host-side routing: all rows of expert k go to core k
    order = np.argsort(rm, kind="stable")
    counts = np.bincount(rm, minlength=E)
    m = max(int(counts.max()), 1024)
    npair = m // 1024
    rem = m - npair * 1024
    if rem == 0:
        rcols = 0
    elif rem <= BLK:
        rcols = max(128, ((rem + 127) // 128) * 128)
    else:
        npair += 1
        rcols = 0
    lone = 1 if rcols else 0
    C = npair * 1024 + lone * rcols
    ndbl = (npair + 1) // 2
    csum = np.zeros(E, dtype=np.int64)
    csum[1:] = np.cumsum(counts)[:-1]
    sorted_expert = rm[order]
    pos_sorted = sorted_expert * C + (np.arange(B) - csum[sorted_expert])

    Xp = np.zeros((E * C, D), np.float16)
    Xp[pos_sorted] = X[order].astype(np.float16)

    W16 = [w.astype(np.float16) for w in Ws]

    in_maps = []
    for core in range(E):
        xt = Xp[core * C : (core + 1) * C].T  # [D, C] fp16 view
        # pairs: interleave the two 128-row halves per pair -> [128, 2048]
        parts = [
            xt[:, : npair * 1024]
            .reshape(2, 128, npair, 2 * BLK)
            .transpose(1, 2, 0, 3)
            .reshape(128, npair * 4 * BLK)
        ]
        if lone:
            xlh = xt[:, npair * 1024 :].reshape(2, 128, rcols)
            parts.append(xlh[0])
            parts.append(xlh[1])
        xint = np.ascontiguousarray(np.concatenate(parts, axis=1))

        wh = np.zeros((128, WCOLS), np.float16)
        wh[:, 0:H] = W16[0][core, 0:128, :]
        wh[:, H : 2 * H] = W16[0][core, 128:256, :]
        for li in range(4):
            wc = 128 + li * 128
            wh[0:64, wc : wc + H] = W16[li + 1][core]
            wh[64:128, wc + H : wc + 128] = W16[li + 1][core]
        wh[0:64, 640 : 640 + A] = W16[5][core]
        wh[64:128, 672 : 672 + A] = W16[5][core]

        bh = np.zeros((128, 6), np.float32)
        for li in range(5):
            bh[0:64, li] = bs[li][core]
            bh[64:128, li] = bs[li][core]
        for r0 in (0, 32, 64, 96):
            bh[r0 : r0 + A, 5] = bs[5][core]

        in_maps.append({"xall": xint, "wt": wh, "bias": bh})

    meta = dict(
        B=B,
        C=C,
        npair=npair,
        rcols=rcols,
        lone=lone,
        ndbl=ndbl,
        order=order,
        pos_sorted=pos_sorted,
    )
    return in_maps, meta


def _finalize(results, meta):
    """results: list (per core) of dicts with 'yt' [128, ycols] fp16."""
    B, C, npair, rcols, lone, ndbl = (
        meta[k] for k in ("B", "C", "npair", "rcols", "lone", "ndbl")
    )
    Yp = np.zeros((E * C, A), np.float32)
    for core in range(E):
        ytc = results[core]["yt"].astype(np.float32)
        for g in range(ndbl):
            cols = slice(g * BLK, (g + 1) * BLK)
            for k, q in enumerate((2 * g, 2 * g + 1)):
                if q >= npair:
                    continue
                dst = core * C + 2 * q * BLK
                r0 = 64 * k
                Yp[dst : dst + BLK] = ytc[r0 : r0 + A, cols].T
                Yp[dst + BLK : dst + 2 * BLK] = ytc[r0 + 32 : r0 + 32 + A, cols].T
        if lone:
            cols = slice(ndbl * BLK, ndbl * BLK + rcols)
            dst = core * C + npair * 1024
            Yp[dst : dst + rcols] = ytc[0:A, cols].T

    y = np.zeros((B, A), np.float32)
    y[meta["order"]] = Yp[meta["pos_sorted"]]
    return y


def kernel(state, rm_state, W1, b1, W2, b2, W3, b3, W4, b4, W5, b5, W6, b6):
    global LAST_RESULTS
    from concourse.bass_utils import run_bass_kernel_spmd

    in_maps, meta = _prepare(
        state, rm_state, W1, b1, W2, b2, W3, b3, W4, b4, W5, b5, W6, b6
    )
    nc = _get_program(meta["npair"], meta["rcols"])
    trace = bool(os.environ.get("KERNEL_TRACE"))
    res = run_bass_kernel_spmd(nc, in_maps, core_ids=list(range(NCORES)), trace=trace)
    LAST_RESULTS = res
    return _finalize(res.results, meta)


# revision 8
# speedup vs baseline: 1.6394x; 1.0780x over previous
"""MoE-routed DeepQNetwork kernel for 8x Trainium2 NeuronCores.

Problem: B=65536 rows, each routed to one of E=8 expert MLPs
(256 -> 64 -> 64 -> 64 -> 64 -> 64 -> 18, ReLU between layers).

Strategy v9 (expert-per-core, staggered wavefront, warm PE, lean acts):
  E == NCORES and the routing is near-uniform (~8192 rows/expert), so core k
  owns ALL rows of expert k, padded to npair 1024-row pairs plus an optional
  short remainder block of r <= 512 rows.  Every core runs the same static
  program with a SINGLE expert's weights (~180 KB).

  Measured constraints driving the design (from perfetto traces):
  - exec_time ~= (last output-store data lands) + ~2.8us: the program
    epilogue is a fixed ~8.7us after the last store and the first ~5.9us
    of preamble is excluded, so the objective is purely to finish the
    last store early.
  - The PE idles at 1.2 GHz until a full ~3.4us HAM activity window is
    busy.  Contraction-1 dummy matmuls do NOT count as activity (only
    one array quadrant lights up) -- the warm-up burst must use full
    K=128 matmuls, bridging program start to the first x-chunk
    semaphore so real matmuls run at 2.4 GHz.
  - Activations (PSUM fp32 -> SBUF fp16 on DVE/ACT, 1 col/cycle, no
    16-bit PSUM on TRN2) are the second resource wall (~12us/engine);
    each instruction also costs ~200-300ns fixed, so stages drain with
    ONE FD-1024 act per two-pair unit (a single reader of both PSUM
    banks).  Only the tail runs finer 256-col segments, placed in
    DIFFERENT banks of one tile so DVE and ACT may drain them in
    parallel (same-bank concurrent access corrupts PSUM reads --
    observed, and documented as fatal).
  - x streams at ~400 GB/s on the sync HWDGE ring; each chunk's
    completion semaphore lags its data by ~2-4us (HBM receipt under
    load), so the last x sem fires ~23us in.  The staggered wavefront
    (stage s of unit u in wave u+s-1, deepest first within a wave)
    keeps independent units in flight; after the last chunk only the
    tail units' act->mm chains remain, shortened by the 256-col splits.
  - Output stores ride the scalar HWDGE ring (gpsimd SWDGE stores add
    Pool-drain + DMASW waits to the epilogue).

  Host: unsort the fp16 outputs back to row order, cast to fp32.
"""

import math
import os

import numpy as np

E = 8
D = 256
H = 64
A = 18
NCORES = 8
BLK = 512  # rows per full block (matmul free dim / PSUM bank cols)
NWARM = 7  # gap-free full-K PE warm-up matmuls (~3us) for the HAM boost

# per-core weight tile [128, WCOLS] fp16 column layout:
#   [0:64)    W1 chunk0 (input dims 0:128)
#   [64:128)  W1 chunk1 (input dims 128:256)
#   [128+128*li : 256+128*li) for li in 0..3: layer 2+li block-diag [128,128]
#             ([0:64,0:64] = W, [64:128,64:128] = W)
#   [640:704) W6 block-diag: [0:64, 0:18] = W6, [64:128, 32:50] = W6
WCOLS = 704

_PROGRAM_CACHE: dict = {}
LAST_RESULTS = None  # test harness can read timing/profile info from here


def _build_program(npair: int, rcols: int):
    """SPMD bass program: npair 1024-row pairs + optional rcols remainder."""
    import concourse.mybir as mybir
    import concourse.tile as tile
    from concourse import bacc

    f32 = mybir.dt.float32
    f16 = mybir.dt.float16
    Relu = mybir.ActivationFunctionType.Relu
    add = mybir.AluOpType.add
    amax = mybir.AluOpType.max

    lone = 1 if rcols else 0
    ndbl = (npair + 1) // 2  # two-pair groups (last may hold one pair)

    nc = bacc.Bacc("TRN2")
    xall = nc.declare_dram_parameter(
        "xall", [128, npair * 2048 + lone * 2 * rcols], f16, isOutput=False
    )
    wt = nc.declare_dram_parameter("wt", [128, WCOLS], f16, isOutput=False)
    # bias cols 0:5 = b1..b5 (rows 0:64 == rows 64:128); col 5 = b6 at rows
    # 0:18 / 32:50 / 64:82 / 96:114
    bias = nc.declare_dram_parameter("bias", [128, 6], f32, isOutput=False)
    yt = nc.declare_dram_parameter(
        "yt", [128, ndbl * BLK + lone * rcols], f16, isOutput=True
    )

    eng_debt = [0.0, 0.0]  # [DVE, ACT] static act load balancer

    def dbl_pairs(d):
        return [q for q in (2 * d, 2 * d + 1) if q < npair]

    tail_dbl = ndbl - 1  # last two-pair unit: 256-col bank-split segments
    tail_pairs = set(dbl_pairs(tail_dbl))

    # tail PSUM tiles are [128,1024] with segment j at columns
    # [j*512, j*512+256): one segment per 512-col bank, so two engines
    # can drain the two segments concurrently.
    TSEG = ((0, 0, 256), (1, 256, 512))  # (bank, row0, row1) row=batch cols

    with tile.TileContext(nc) as tc:
        with (
            tc.tile_pool(name="wpool", bufs=1) as wpool,
            tc.tile_pool(name="xpool", bufs=2) as xpool,
            tc.tile_pool(name="hpool", bufs=1) as hpool,
            tc.tile_pool(name="opool", bufs=3) as opool,
            # PSUM budget (8 banks): pA 1 x [128,1024] (warm-ups, L1, L6)
            # + pB 3 x [128,1024] (mid layers)
            tc.tile_pool(name="pA", bufs=1, space="PSUM") as pApool,
            tc.tile_pool(name="pB", bufs=3, space="PSUM") as pBpool,
        ):
            # ---- PE warm-up source (memset, no DMA dependence)
            warm_src = wpool.tile([128, 640], f16, name="warm_src", tag="ws", bufs=1)
            nc.vector.memset(warm_src[:, :], 0.0)

            # ---- DMA issue.  Weights+bias on the scalar HW-DGE ring
            # first; all x on the sync ring in consumption order; output
            # stores ride the scalar ring later.
            w_sb = wpool.tile([128, WCOLS], f16, name="w_sb", tag="w", bufs=1)
            nc.scalar.dma_start(out=w_sb[:, :], in_=wt[:, :])
            bias_sb = wpool.tile([128, 6], f32, name="bias_sb", tag="bias", bufs=1)
            nc.scalar.dma_start(out=bias_sb[:, :], in_=bias[:, :])

            # x chunks: pair0 as two 1024-col halves, pair1, then 1MB
            # two-pair chunks; the tail dbl's pairs and the remainder get
            # their own chunks (finer completion sems for the drain).
            p0 = []
            for i in (0, 1):
                t = xpool.tile([128, 1024], f16, tag=f"x0h{i}", name=f"x0h{i}", bufs=1)
                nc.sync.dma_start(out=t[:, :], in_=xall[:, i * 1024 : (i + 1) * 1024])
                p0.append(t)
            xc1 = None
            if npair > 1:
                xc1 = xpool.tile([128, 2048], f16, tag="xc1", name="xc_1", bufs=1)
                nc.sync.dma_start(out=xc1[:, :], in_=xall[:, 2048:4096])
            xds: dict = {}
            for dd in range(1, ndbl):
                prs = dbl_pairs(dd)
                if dd == tail_dbl:
                    for q in prs:
                        t = xpool.tile(
                            [128, 2048], f16, tag=f"xp{q}", name=f"xp_{q}", bufs=1
                        )
                        nc.sync.dma_start(
                            out=t[:, :], in_=xall[:, q * 2048 : (q + 1) * 2048]
                        )
                        xds[q] = t
                else:
                    w = len(prs) * 2048
                    t = xpool.tile([128, w], f16, tag=f"xd{dd}", name=f"xd_{dd}", bufs=1)
                    nc.sync.dma_start(
                        out=t[:, :], in_=xall[:, 2 * dd * 2048 : 2 * dd * 2048 + w]
                    )
                    for q in prs:
                        xds[q] = (t, (q % 2) * 2048)
            xl = None
            if lone:
                xl = xpool.tile([128, 2 * rcols], f16, tag="xl", name="xlone", bufs=1)
                nc.sync.dma_start(
                    out=xl[:, :],
                    in_=xall[:, npair * 2048 : npair * 2048 + 2 * rcols],
                )

            def x_rhs(p, blk, c, c0, c1):
                # columns [c0:c1) of contraction chunk c of block blk of pair p
                lo = c * 1024 + blk * BLK + c0
                if p == 0:
                    return p0[c][:, blk * BLK + c0 : blk * BLK + c1]
                if p == 1:
                    return xc1[:, lo : lo + (c1 - c0)]
                ent = xds[p]
                if isinstance(ent, tuple):
                    t, off = ent
                    return t[:, off + lo : off + lo + (c1 - c0)]
                return ent[:, lo : lo + (c1 - c0)]

            # ---- PE warm-up burst: full-K matmuls (gap-free, never read)
            # so the HAM activity monitor sees the whole array busy.
            for i in range(NWARM):
                wps = pApool.tile([128, 1024], f32, tag="pA", name=f"warm_{i}")
                nc.tensor.matmul(
                    out=wps[:, 0:BLK],
                    lhsT=warm_src[:, 0:128],
                    rhs=warm_src[:, 128:640],
                    start=True,
                    stop=True,
                )

            def act(out_ap, in_ap, bias_ap, relu, fd, force=None):
                cost_v = (120.0 + fd) / 0.96 + 250.0
                cost_s = (172.0 + fd) / 1.2 + 250.0
                use_v = (
                    force == 0
                    if force is not None
                    else eng_debt[0] + cost_v <= eng_debt[1] + cost_s
                )
                if use_v:
                    eng_debt[0] += cost_v
                    if relu:
                        nc.vector.tensor_scalar(
                            out_ap, in_ap, bias_ap, 0.0, op0=add, op1=amax
                        )
                    else:
                        nc.vector.tensor_scalar(out_ap, in_ap, bias_ap, None, op0=add)
                else:
                    eng_debt[1] += cost_s
                    if relu:
                        nc.scalar.activation(out_ap, in_ap, Relu, bias=bias_ap)
                    else:
                        nc.scalar.add(out_ap, in_ap, bias_ap)

            # h tiles: per-unit [128,1024] for non-tail dbls; per
            # (pair, seg) [128,256] for tail pairs; [64,rcols] for lone
            h1u: dict = {}
            hmu: dict = {}

            def l1_mms(p, ph, co, c0, c1):
                # (chunk, block)-ordered so the two 64-wide column-group
                # matmuls of a pair stream concurrently
                for c in (0, 1):
                    for blk, colr in ((0, slice(0, 64)), (1, slice(64, 128))):
                        nc.tensor.matmul(
                            out=ph[colr, co + c0 : co + c1],
                            lhsT=w_sb[:, c * H : (c + 1) * H],
                            rhs=x_rhs(p, blk, c, c0, c1),
                            start=(c == 0),
                            stop=(c == 1),
                        )

            def emit_s1(u):
                if u == -1:
                    ph = pApool.tile([128, 1024], f32, tag="pA", name="ph1_L")
                    for c in (0, 1):
                        nc.tensor.matmul(
                            out=ph[0:64, 0:rcols],
                            lhsT=w_sb[:, c * H : (c + 1) * H],
                            rhs=xl[:, c * rcols : (c + 1) * rcols],
                            start=(c == 0),
                            stop=(c == 1),
                        )
                    hl = hpool.tile([64, rcols], f16, tag="h1L", name="h1_L", bufs=1)
                    act(hl[:, :], ph[0:64, 0:rcols], bias_sb[0:64, 0:1], True, rcols)
                    h1u[("L", 0)] = hl
                    return
                pairs = dbl_pairs(u)
                if u == tail_dbl:
                    for p in pairs:
                        ph = pApool.tile([128, 1024], f32, tag="pA", name=f"ph1_{p}")
                        for j, s0, s1 in TSEG:
                            l1_mms(p, ph, j * BLK - s0, s0, s1)
                        for j, s0, s1 in TSEG:
                            h1 = hpool.tile(
                                [128, s1 - s0],
                                f16,
                                tag=f"h1_{p}_{j}",
                                name=f"h1_{p}_{j}",
                                bufs=1,
                            )
                            act(
                                h1[:, :],
                                ph[:, j * BLK : j * BLK + (s1 - s0)],
                                bias_sb[:, 0:1],
                                True,
                                s1 - s0,
                                force=j,
                            )
                            h1u[(p, j)] = h1
                else:
                    ph = pApool.tile([128, 1024], f32, tag="pA", name=f"ph1u_{u}")
                    for k, p in enumerate(pairs):
                        l1_mms(p, ph, k * BLK, 0, BLK)
                    w = len(pairs) * BLK
                    h1 = hpool.tile([128, w], f16, tag=f"h1u_{u}", name=f"h1u_{u}", bufs=1)
                    act(h1[:, :], ph[:, 0:w], bias_sb[:, 0:1], True, w)
                    h1u[u] = h1

            def emit_mid(li, u):
                wc = 128 + (li - 2) * 128
                if u == -1:
                    prev = h1u[("L", 0)] if li == 2 else hmu[(li - 1, "L", 0)]
                    ph = pBpool.tile([128, 1024], f32, tag="pB", name=f"ph{li}_L")
                    nc.tensor.matmul(
                        out=ph[0:64, 0:rcols],
                        lhsT=w_sb[0:64, wc : wc + 64],
                        rhs=prev[:, :],
                        start=True,
                        stop=True,
                    )
                    hl = hpool.tile(
                        [64, rcols], f16, tag=f"h{li}L", name=f"h{li}_L", bufs=1
                    )
                    act(
                        hl[:, :],
                        ph[0:64, 0:rcols],
                        bias_sb[0:64, li - 1 : li],
                        True,
                        rcols,
                    )
                    hmu[(li, "L", 0)] = hl
                    return
                pairs = dbl_pairs(u)
                if u == tail_dbl:
                    for p in pairs:
                        ph = pBpool.tile(
                            [128, 1024], f32, tag="pB", name=f"ph{li}_{p}"
                        )
                        for j, s0, s1 in TSEG:
                            w = s1 - s0
                            prev = h1u[(p, j)] if li == 2 else hmu[(li - 1, p, j)]
                            nc.tensor.matmul(
                                out=ph[:, j * BLK : j * BLK + w],
                                lhsT=w_sb[:, wc : wc + 128],
                                rhs=prev[:, :],
                                start=True,
                                stop=True,
                            )
                        for j, s0, s1 in TSEG:
                            w = s1 - s0
                            h = hpool.tile(
                                [128, w],
                                f16,
                                tag=f"h{li}_{p}_{j}",
                                name=f"h{li}_{p}_{j}",
                                bufs=1,
                            )
                            act(
                                h[:, :],
                                ph[:, j * BLK : j * BLK + w],
                                bias_sb[:, li - 1 : li],
                                True,
                                w,
                                force=j,
                            )
                            hmu[(li, p, j)] = h
                else:
                    prev = h1u[u] if li == 2 else hmu[(li - 1, u)]
                    ph = pBpool.tile([128, 1024], f32, tag="pB", name=f"ph{li}u_{u}")
                    w = len(pairs) * BLK
                    for k, p in enumerate(pairs):
                        nc.tensor.matmul(
                            out=ph[:, k * BLK : (k + 1) * BLK],
                            lhsT=w_sb[:, wc : wc + 128],
                            rhs=prev[:, k * BLK : (k + 1) * BLK],
                            start=True,
                            stop=True,
                        )
                    h = hpool.tile(
                        [128, w], f16, tag=f"h{li}u_{u}", name=f"h{li}u_{u}", bufs=1
                    )
                    act(h[:, :], ph[:, 0:w], bias_sb[:, li - 1 : li], True, w)
                    hmu[(li, u)] = h

            def emit_s6(u):
                # L6 [64 -> 18]: pairs 2u / 2u+1 -> PSUM rows 0:64 / 64:128
                # (concurrent column groups); store per act on scalar ring
                if u == -1:
                    po = pApool.tile([128, 1024], f32, tag="pA", name="po_L")
                    nc.tensor.matmul(
                        out=po[0:32, 0:rcols],
                        lhsT=w_sb[0:64, 640:672],
                        rhs=hmu[(5, "L", 0)][:, :],
                        start=True,
                        stop=True,
                    )
                    o = opool.tile([32, rcols], f16, tag="og", name="o_L")
                    act(o[:, :], po[0:32, 0:rcols], bias_sb[0:32, 5:6], False, rcols)
                    nc.scalar.dma_start(
                        out=yt[0:32, ndbl * BLK : ndbl * BLK + rcols], in_=o[:, :]
                    )
                    return
                pairs = dbl_pairs(u)
                rows = 64 * len(pairs)
                po = pApool.tile([128, 1024], f32, tag="pA", name=f"po_{u}")
                if u == tail_dbl:
                    for j, s0, s1 in TSEG:
                        w = s1 - s0
                        for k, q in enumerate(pairs):
                            nc.tensor.matmul(
                                out=po[64 * k : 64 * (k + 1), j * BLK : j * BLK + w],
                                lhsT=w_sb[:, 640:704],
                                rhs=hmu[(5, q, j)][:, :],
                                start=True,
                                stop=True,
                            )
                    for j, s0, s1 in TSEG:
                        w = s1 - s0
                        o = opool.tile([rows, w], f16, tag="og", name=f"o_{u}_{j}")
                        act(
                            o[:, :],
                            po[0:rows, j * BLK : j * BLK + w],
                            bias_sb[0:rows, 5:6],
                            False,
                            w,
                            force=j,
                        )
                        nc.scalar.dma_start(
                            out=yt[0:rows, u * BLK + s0 : u * BLK + s1], in_=o[:, :]
                        )
                else:
                    for k, q in enumerate(pairs):
                        nc.tensor.matmul(
                            out=po[64 * k : 64 * (k + 1), 0:BLK],
                            lhsT=w_sb[:, 640:704],
                            rhs=hmu[(5, u)][:, k * BLK : (k + 1) * BLK],
                            start=True,
                            stop=True,
                        )
                    o = opool.tile([rows, BLK], f16, tag="og", name=f"o_{u}")
                    act(o[:, :], po[0:rows, 0:BLK], bias_sb[0:rows, 5:6], False, BLK)
                    nc.scalar.dma_start(
                        out=yt[0:rows, u * BLK : (u + 1) * BLK], in_=o[:, :]
                    )

            def emit_stage(s, u):
                if s == 1:
                    emit_s1(u)
                elif s == 6:
                    emit_s6(u)
                else:
                    emit_mid(s, u)

            # ---- staggered wavefront: stage s of unit i in wave i+s-1,
            # deepest stage first within each wave (oldest dependencies),
            # so an x-sem wait at the wave's trailing L1 never starves the
            # PE of ready deep-layer work.
            units = list(range(ndbl)) + ([-1] if lone else [])
            nunits = len(units)
            lag = {1: 0, 2: 1, 3: 2, 4: 3, 5: 4, 6: 5}
            for wave in range(nunits + lag[6]):
                for s in (6, 5, 4, 3, 2, 1):
                    i = wave - lag[s]
                    if 0 <= i < nunits:
                        emit_stage(s, units[i])

    nc.compile()
    return nc


def _get_program(npair: int, rcols: int):
    key = (npair, rcols)
    if key not in _PROGRAM_CACHE:
        _PROGRAM_CACHE[key] = _build_program(npair, rcols)
    return _PROGRAM_CACHE[key]


def _prepare(state, rm_state, W1, b1, W2, b2, W3, b3, W4, b4, W5, b5, W6, b6):
    state = np.ascontiguousarray(np.asarray(state, dtype=np.float32))
    rm = np.asarray(rm_state).reshape(-1).astype(np.int64)
    Ws = [np.asarray(w, dtype=np.float32) for w in (W1, W2, W3, W4, W5, W6)]
    bs = [np.asarray(b, dtype=np.float32) for b in (b1, b2, b3, b4, b5, b6)]
    B = state.shape[0]
    X = state.reshape(B, D)

    # ---- host-side routing: all rows of expert k go to core k
    order = np.argsort(rm, kind="stable")
    counts = np.bincount(rm, minlength=E)
    m = max(int(counts.max()), 1024)
    npair = m // 1024
    rem = m - npair * 1024
    if rem == 0:
        rcols = 0
    elif rem <= BLK:
        rcols = max(128, ((rem + 127) // 128) * 128)
    else:
        npair += 1
        rcols = 0
    lone = 1 if rcols else 0
    C = npair * 1024 + lone * rcols
    ndbl = (npair + 1) // 2
    csum = np.zeros(E, dtype=np.int64)
    csum[1:] = np.cumsum(counts)[:-1]
    sorted_expert = rm[order]
    pos_sorted = sorted_expert * C + (np.arange(B) - csum[sorted_expert])

    Xp = np.zeros((E * C, D), np.float16)
    Xp[pos_sorted] = X[order].astype(np.float16)

    W16 = [w.astype(np.float16) for w in Ws]

    in_maps = []
    for core in range(E):
        xt = Xp[core * C : (core + 1) * C].T  # [D, C] fp16 view
        # pairs: interleave the two 128-row halves per pair -> [128, 2048]
        parts = [
            xt[:, : npair * 1024]
            .reshape(2, 128, npair, 2 * BLK)
            .transpose(1, 2, 0, 3)
            .reshape(128, npair * 4 * BLK)
        ]
        if lone:
            xlh = xt[:, npair * 1024 :].reshape(2, 128, rcols)
            parts.append(xlh[0])
            parts.append(xlh[1])
        xint = np.ascontiguousarray(np.concatenate(parts, axis=1))

        wh = np.zeros((128, WCOLS), np.float16)
        wh[:, 0:H] = W16[0][core, 0:128, :]
        wh[:, H : 2 * H] = W16[0][core, 128:256, :]
        for li in range(4):
            wc = 128 + li * 128
            wh[0:64, wc : wc + H] = W16[li + 1][core]
            wh[64:128, wc + H : wc + 128] = W16[li + 1][core]
        wh[0:64, 640 : 640 + A] = W16[5][core]
        wh[64:128, 672 : 672 + A] = W16[5][core]

        bh = np.zeros((128, 6), np.float32)
        for li in range(5):
            bh[0:64, li] = bs[li][core]
            bh[64:128, li] = bs[li][core]
        for r0 in (0, 32, 64, 96):
            bh[r0 : r0 + A, 5] = bs[5][core]

        in_maps.append({"xall": xint, "wt": wh, "bias": bh})

    meta = dict(
        B=B,
        C=C,
        npair=npair,
        rcols=rcols,
        lone=lone,
        ndbl=ndbl,
        order=order,
        pos_sorted=pos_sorted,
    )
    return in_maps, meta


def _finalize(results, meta):
    """results: list (per core) of dicts with 'yt' [128, ycols] fp16."""
    B, C, npair, rcols, lone, ndbl = (
        meta[k] for k in ("B", "C", "npair", "rcols", "lone", "ndbl")
    )
    Yp = np.zeros((E * C, A), np.float32)
    for core in range(E):
        ytc = results[core]["yt"].astype(np.float32)
        for g in range(ndbl):
            cols = slice(g * BLK, (g + 1) * BLK)
            for k, q in enumerate((2 * g, 2 * g + 1)):
                if q >= npair:
                    continue
                dst = core * C + 2 * q * BLK
                r0 = 64 * k
                Yp[dst : dst + BLK] = ytc[r0 : r0 + A, cols].T
                Yp[dst + BLK : dst + 2 * BLK] = ytc[r0 + 32 : r0 + 32 + A, cols].T
        if lone:
            cols = slice(ndbl * BLK, ndbl * BLK + rcols)
            dst = core * C + npair * 1024
            Yp[dst : dst + rcols] = ytc[0:A, cols].T

    y = np.zeros((B, A), np.float32)
    y[meta["order"]] = Yp[meta["pos_sorted"]]
    return y


def kernel(state, rm_state, W1, b1, W2, b2, W3, b3, W4, b4, W5, b5, W6, b6):
    global LAST_RESULTS
    from concourse.bass_utils import run_bass_kernel_spmd

    in_maps, meta = _prepare(
        state, rm_state, W1, b1, W2, b2, W3, b3, W4, b4, W5, b5, W6, b6
    )
    nc = _get_program(meta["npair"], meta["rcols"])
    trace = bool(os.environ.get("KERNEL_TRACE"))
    res = run_bass_kernel_spmd(nc, in_maps, core_ids=list(range(NCORES)), trace=trace)
    LAST_RESULTS = res
    return _finalize(res.results, meta)
